# revision 1
# baseline (speedup 1.0000x reference)
"""Single-head causal self-attention (B=4, T=4096, C=1024, HS=64) on 8 TRN2 cores.

Sharding: core = 2*b + h; the two cores of batch b split the 8 query blocks
(512 rows each) in a load-balanced interleave: h=0 -> blocks {0,3,4,7},
h=1 -> blocks {1,2,5,6} (equal causal-score work: 80 context chunks each).

The SPMD program is identical on every core; per-core differences are pure
data:
  xt  = x[b].T (shared context, global order)
  xtq = x[b, blocks].T (the core's query rows, gathered host-side)
  thr = causal-mask threshold columns (position-aware, per core)
Slot j processes query block g_j against context prefix [0, 128*NCH[j]);
the last 8 context chunks of each slot are masked with data-driven
thresholds against a ramp constant (handles the diagonal, "future" rows
inside the uniform prefix, and fully-masked padding chunks alike).

Dataflow per core (matmul operands bf16, PSUM f32):
  A1: [K^T|V^T] tiles = ([Wk | Wv]).T @ xt      (N=1024 moving, 8 c-chunks)
      V^T -> PE-transpose -> V natural, ones column appended (softmax sums)
  A2: Q^T = (Wq/8).T @ xtq
  C:  S^T piece = K^T_chunk.T @ Q^T_piece        (K-dim = 64, N<=1024)
      E = exp(S^T) (ScalarE, psum->sbuf bf16), mask via precomputed tiles
  D:  O^T[65, q] += [V|1]_chunk.T @ E_piece      (row 64 = softmax sums)
  E:  PE-transpose O^T -> O, out = O[:, :64] * (1 / O[:, 64])
"""

import numpy as np
import ml_dtypes

B, T, C, HS = 4, 4096, 1024, 64
QH = T // 2            # queries per core
NSLOT = 4
NCH = [8, 16, 24, 32]  # uniform context chunks (of 128) per slot
BLOCKS = [[0, 3, 4, 7], [1, 2, 5, 6]]  # global 512-blocks per half
CCH = C // 128

_compiled = None


def _build_program():
    import concourse.bass as bass
    import concourse.mybir as mybir
    import concourse.tile as tile
    from concourse import bacc
    from concourse.masks import make_identity
    from contextlib import ExitStack

    f32 = mybir.dt.float32
    bf16 = mybir.dt.bfloat16

    nc = bacc.Bacc("TRN2", target_bir_lowering=False, debug=False, num_devices=8)

    xt_d = nc.dram_tensor("xt", [C, T], bf16, kind="ExternalInput").ap()
    xtq_d = nc.dram_tensor("xtq", [C, QH], bf16, kind="ExternalInput").ap()
    wkv_d = nc.dram_tensor("wkv", [C, 128], bf16, kind="ExternalInput").ap()
    wq_d = nc.dram_tensor("wq", [C, HS], bf16, kind="ExternalInput").ap()
    ramp_d = nc.dram_tensor("ramp", [128, 512], f32, kind="ExternalInput").ap()
    thr_d = nc.dram_tensor("thr", [128, 32], f32, kind="ExternalInput").ap()
    out_d = nc.dram_tensor("out", [QH, HS], f32, kind="ExternalOutput").ap()

    with tile.TileContext(nc) as tc, ExitStack() as ctx:
        consts = ctx.enter_context(tc.tile_pool(name="consts", bufs=1))
        epool = ctx.enter_context(tc.tile_pool(name="epool", bufs=6))
        mpool = ctx.enter_context(tc.tile_pool(name="mpool", bufs=2))
        opool = ctx.enter_context(tc.tile_pool(name="opool", bufs=4))

        xt = consts.tile([128, CCH, T], bf16)
        xtq = consts.tile([128, CCH, QH], bf16)
        wkv = consts.tile([128, CCH, 128], bf16)
        wq = consts.tile([128, CCH, HS], bf16)
        kT = consts.tile([64, T], bf16)
        qT = consts.tile([64, QH], bf16)
        vp = consts.tile([128, T // 128, HS + 1], bf16)  # [V | ones]
        ramp = consts.tile([128, 512], f32)
        thr = consts.tile([128, 32], f32)
        id_bf = consts.tile([64, 64], bf16)
        id_f32 = consts.tile([65, 65], f32)

        nc.sync.dma_start(out=wkv, in_=wkv_d.rearrange("(a p) m -> p a m", p=128))
        nc.sync.dma_start(out=wq, in_=wq_d.rearrange("(a p) m -> p a m", p=128))
        nc.sync.dma_start(out=ramp, in_=ramp_d)
        nc.sync.dma_start(out=thr, in_=thr_d)
        make_identity(nc, id_bf)
        make_identity(nc, id_f32)
        nc.vector.memset(vp[:, :, HS], 1.0)

        # xtq first (A2 unblocks early), then xt; split across HWDGE/SWDGE
        xtq_r = xtq_d.rearrange("(a p) t -> p a t", p=128)
        for tb in range(QH // 512):
            sl = slice(tb * 512, tb * 512 + 512)
            eng = nc.gpsimd if tb % 2 == 0 else nc.sync
            eng.dma_start(out=xtq[:, :, sl], in_=xtq_r[:, :, sl])
        xt_r = xt_d.rearrange("(a p) t -> p a t", p=128)
        for tb in range(T // 512):
            sl = slice(tb * 512, tb * 512 + 512)
            eng = nc.sync if tb % 2 == 0 else nc.gpsimd
            eng.dma_start(out=xt[:, :, sl], in_=xt_r[:, :, sl])

        # precompute the 32 causal-mask tiles on the idle GPSIMD engine
        mk = [consts.tile([128, 512], bf16, name=f"mk_{i}") for i in range(32)]
        for i in range(32):
            nc.gpsimd.tensor_scalar(
                mk[i], ramp, thr[:, i:i + 1], None, op0=mybir.AluOpType.is_ge)

        # ---- single PSUM scope: pa 2 + pc/tr 2 + o_t 4 = 8 banks ----
        ot_all = consts.tile([128, QH // 128, HS], f32)
        with tc.tile_pool(name="psA", bufs=1, space="PSUM") as psA, \
             tc.tile_pool(name="psC", bufs=3, space="PSUM") as psC, \
             tc.tile_pool(name="psD", bufs=4, space="PSUM") as psD:
            for tb in range(QH // 512):   # A2: Q^T over the query rows
                sl = slice(tb * 512, tb * 512 + 512)
                pq = psA.tile([64, 512], f32, tag="pa", name=f"pq_{tb}")
                for ci in range(CCH):
                    nc.tensor.matmul(pq, wq[:, ci, :], xtq[:, ci, sl],
                                     start=(ci == 0), stop=(ci == CCH - 1))
                nc.vector.tensor_copy(qT[:, sl], pq)
            for tb in range(T // 512):    # A1: K^T | V^T over context
                sl = slice(tb * 512, tb * 512 + 512)
                pa = psA.tile([128, 512], f32, tag="pa", name=f"pa_{tb}")
                for ci in range(CCH):
                    nc.tensor.matmul(pa, wkv[:, ci, :], xt[:, ci, sl],
                                     start=(ci == 0), stop=(ci == CCH - 1))
                nc.vector.tensor_copy(kT[:, sl], pa[0:64, :])
                vts = epool.tile([64, 512], bf16, tag="vts", name=f"vts_{tb}")
                nc.vector.tensor_copy(vts, pa[64:128, :])
                for blk in range(4):
                    k = tb * 4 + blk
                    vtp = psA.tile([128, HS], bf16, tag="pa", name=f"vtp_{k}")
                    nc.tensor.transpose(
                        vtp, vts[:, blk * 128:blk * 128 + 128], id_bf)
                    nc.vector.tensor_copy(vp[:, k, 0:HS], vtp)

            # attention: slots round-robin by normalized progress so all
            # four chains stay live to the end (no serial tail)
            o_t = [psD.tile([65, 512], f32, tag="ot", name=f"o_t_{j}")
                   for j in range(NSLOT)]
            sched = []
            prog = [0] * NSLOT
            ends = [26, 28, 30, 32]   # staggered so finalizes overlap work
            for step in range(max(NCH)):
                for j in range(NSLOT - 1, -1, -1):
                    target = min(NCH[j], ((step + 1) * NCH[j] + ends[j] - 1)
                                 // ends[j])
                    while prog[j] < target:
                        sched.append((j, prog[j]))
                        prog[j] += 1
            for j, k in sched:
                ksl = slice(k * 128, k * 128 + 128)
                qsl = slice(j * 512, j * 512 + 512)
                pc = psC.tile([128, 512], f32, tag="pc", name=f"pc_{k}_{j}")
                nc.tensor.matmul(pc, kT[:, ksl], qT[:, qsl],
                                 start=True, stop=True)
                et = epool.tile([128, 512], bf16, tag="et",
                                name=f"et_{k}_{j}")
                nc.scalar.activation(et, pc, mybir.ActivationFunctionType.Exp)
                m = k - (NCH[j] - 8)
                if 0 <= m < 8:
                    nc.vector.tensor_mul(et, et, mk[8 * j + m])
                nc.tensor.matmul(o_t[j], vp[:, k, :], et,
                                 start=(k == 0), stop=(k == NCH[j] - 1))
                if k == NCH[j] - 1:   # finalize slot j now
                    ops = epool.tile([65, 512], f32, tag="ops",
                                     name=f"ops_{j}")
                    nc.vector.tensor_copy(ops, o_t[j])
                    for qs in range(4):
                        tp = psA.tile([128, HS + 1], f32, tag="pa",
                                      name=f"tp_{j}_{qs}")
                        nc.tensor.transpose(
                            tp, ops[:, qs * 128:qs * 128 + 128], id_f32)
                        rec = mpool.tile([128, 1], f32, tag="rec",
                                         name=f"rec_{j}_{qs}")
                        nc.vector.reciprocal(rec, tp[:, HS:HS + 1])
                        nc.vector.tensor_scalar_mul(
                            ot_all[:, 4 * j + qs, :], tp[:, 0:HS], rec)
        nc.sync.dma_start(
            out=out_d.rearrange("(q p) h -> p q h", p=128), in_=ot_all)

    nc.compile()
    return nc


def _prep_inputs(x, Wq, Wk, Wv):
    bf = ml_dtypes.bfloat16
    wkv = np.concatenate([Wk, Wv], axis=1).astype(bf)   # [C, 128]
    wq = (Wq * 0.125).astype(bf)
    ramp = np.broadcast_to(np.arange(512, dtype=np.float32), (128, 512)).copy()
    p = np.arange(128, dtype=np.float32)
    in_maps = []
    for core in range(8):
        b, h = core // 2, core % 2
        blocks = BLOCKS[h]
        xt = np.ascontiguousarray(x[b].T).astype(bf)
        xtq = np.concatenate(
            [x[b, g * 512:(g + 1) * 512] for g in blocks], axis=0
        ).T.astype(bf)
        thr = np.zeros((128, 32), np.float32)
        for j in range(NSLOT):
            for m in range(8):
                kk = NCH[j] - 8 + m
                thr[:, 8 * j + m] = 128 * kk + p - 512 * blocks[j]
        in_maps.append({
            "xt": np.ascontiguousarray(xt),
            "xtq": np.ascontiguousarray(xtq),
            "wkv": wkv, "wq": wq, "ramp": ramp, "thr": thr,
        })
    return in_maps


def kernel(x, Wq, Wk, Wv):
    from concourse.bass_utils import run_bass_kernel_spmd

    global _compiled
    if _compiled is None:
        _compiled = _build_program()
    nc = _compiled

    in_maps = _prep_inputs(
        np.asarray(x, np.float32), np.asarray(Wq, np.float32),
        np.asarray(Wk, np.float32), np.asarray(Wv, np.float32),
    )
    res = run_bass_kernel_spmd(nc, in_maps, list(range(8)))
    out = np.empty((B, T, HS), np.float32)
    for core in range(8):
        b, h = core // 2, core % 2
        o = res.results[core]["out"]
        for j, g in enumerate(BLOCKS[h]):
            out[b, g * 512:(g + 1) * 512] = o[j * 512:(j + 1) * 512]
    return out


if __name__ == "__main__":
    rng = np.random.default_rng(0)
    x = rng.standard_normal((B, T, C), dtype=np.float32)
    s = 1 / np.sqrt(C)
    Wq = rng.standard_normal((C, HS), dtype=np.float32) * s
    Wk = rng.standard_normal((C, HS), dtype=np.float32) * s
    Wv = rng.standard_normal((C, HS), dtype=np.float32) * s
    o = kernel(x=x, Wq=Wq, Wk=Wk, Wv=Wv)
    print(o.shape, o.dtype, np.abs(o).mean())



# revision 4
# speedup vs baseline: 1.1663x; 1.1663x over previous
"""Single-head causal self-attention (B=4, T=4096, C=1024, HS=64) on 8 TRN2 cores.

Sharding: core = 2*b + h; the two cores of batch b split the 8 query blocks
(512 rows each) in a load-balanced interleave (h=0 -> {0,3,4,7}, h=1 ->
{1,2,5,6}; 80 causal context chunks each).

The SPMD program is identical on every core; per-core differences are pure
data. Each core's context x[b] is PERMUTED host-side at 512-block granularity
so that the core's own query blocks sit at fixed program positions 0,2,4,6,
while every slot's causal context prefix is covered by the first 2(j+1)
permuted blocks. Causal-mask thresholds (per-core data) absorb the
permutation.

Dataflow per core (wavefront over 8 context blocks tb, matmuls bf16/fp8):
  A1(tb): [K^T|V^T] = [Wk|Wv]^T @ xt_tb   (PSUM [128,512], 8 c-chunks)
          K^T rows -> fp8 [32,2,T] split layout (DoubleRow operands)
          V^T -> PE-transpose -> V natural [128k, 64], ones col appended
  A2(tb even): Q^T slot j=tb/2 from xt block tb -> fp8 [32,2,QH]
  attention pairs (j, p) drained slot-major behind the wavefront:
    C: S^T pair [128k, 2x512q] = K^T.T @ Q^T  (fp8 DoubleRow, PSUM 2 banks)
    E: et = exp(0.125 * S^T) (ScalarE, one op per pair), mask last 4 pairs
    D: O[128q, 65] += et_chunk_qslice.T @ [V|1]_chunk  (65-wide moving side)
  finalize slot: rec = 1/O[:,64]; out = O[:,0:64]*rec; per-slot DMA out
"""

import numpy as np
import ml_dtypes

B, T, C, HS = 4, 4096, 1024, 64
QH = T // 2            # queries per core
NSLOT = 4
NCH = [8, 16, 24, 32]  # uniform context chunks (of 128) per slot
CCH = C // 128
BLOCKS = [[0, 3, 4, 7], [1, 2, 5, 6]]  # own query blocks per half
# permuted context layout: own blocks at positions 0,2,4,6; prefix-coverage
# of each slot's causal context holds for both halves
PERM = [[0, 1, 3, 2, 4, 5, 7, 6], [1, 0, 2, 3, 5, 4, 6, 7]]

USE_FP8 = False

_compiled = None


def _attn_schedule():
    """(j, p) pair emission order: slot-major, gated by wavefront tb."""
    work = []
    for j in range(NSLOT):
        for p in range(NCH[j] // 2):
            work.append((j, p))
    sched = [[] for _ in range(8)]
    ptr = 0
    for tb in range(8):
        while ptr < len(work):
            j, p = work[ptr]
            if tb >= 2 * j and p <= 2 * tb + 1:
                sched[tb].append((j, p))
                ptr += 1
            else:
                break
    assert ptr == len(work)
    return sched


def _build_program():
    import concourse.bass as bass
    import concourse.mybir as mybir
    import concourse.tile as tile
    from concourse import bacc
    from concourse.masks import make_identity
    from contextlib import ExitStack

    f32 = mybir.dt.float32
    bf16 = mybir.dt.bfloat16
    fp8 = mybir.dt.float8e4
    cdt = fp8 if USE_FP8 else bf16

    nc = bacc.Bacc("TRN2", target_bir_lowering=False, debug=False, num_devices=8)

    xt_d = nc.dram_tensor("xt", [C, T], bf16, kind="ExternalInput").ap()
    wkv_d = nc.dram_tensor("wkv", [C, 128], bf16, kind="ExternalInput").ap()
    wq_d = nc.dram_tensor("wq", [C, HS], bf16, kind="ExternalInput").ap()
    ramp2_d = nc.dram_tensor("ramp2", [128, 1024], f32, kind="ExternalInput").ap()
    thr2_d = nc.dram_tensor("thr2", [128, 16], f32, kind="ExternalInput").ap()
    out_d = nc.dram_tensor("out", [QH, HS], f32, kind="ExternalOutput").ap()

    sched = _attn_schedule()

    with tile.TileContext(nc) as tc, ExitStack() as ctx:
        consts = ctx.enter_context(tc.tile_pool(name="consts", bufs=1))
        epool = ctx.enter_context(tc.tile_pool(name="epool", bufs=4))
        mpool = ctx.enter_context(tc.tile_pool(name="mpool", bufs=2))

        xt = consts.tile([128, CCH, T], bf16)
        wkv = consts.tile([128, CCH, 128], bf16)
        wq = consts.tile([128, CCH, HS], bf16)
        ramp2 = consts.tile([128, 2, 512], f32)
        thr2 = consts.tile([128, 16], f32)
        id_bf = consts.tile([64, 64], bf16)
        if USE_FP8:
            kTv = consts.tile([32, 2, T], cdt)
            qTv = consts.tile([32, 2, QH], cdt)
        else:
            kTv = consts.tile([64, T], cdt)
            qTv = consts.tile([64, QH], cdt)
        vp = consts.tile([128, T // 128, HS + 1], bf16)  # [V | ones]
        outs = consts.tile([128, QH // 128, HS], f32)

        nc.sync.dma_start(out=wkv, in_=wkv_d.rearrange("(a p) m -> p a m", p=128))
        nc.sync.dma_start(out=wq, in_=wq_d.rearrange("(a p) m -> p a m", p=128))
        nc.sync.dma_start(out=ramp2, in_=ramp2_d.rearrange("p (d q) -> p d q", d=2))
        nc.sync.dma_start(out=thr2, in_=thr2_d)
        make_identity(nc, id_bf)
        nc.vector.memset(vp[:, :, HS], 1.0)

        # context granules, 256 cols each, in wavefront order
        xt_r = xt_d.rearrange("(a p) t -> p a t", p=128)
        for g in range(T // 256):
            sl = slice(g * 256, g * 256 + 256)
            nc.sync.dma_start(out=xt[:, :, sl], in_=xt_r[:, :, sl])

        # mask pairs on the idle GPSIMD engine, slot-major so slot 0's are
        # ready first
        mk = [consts.tile([128, 2, 512], bf16, name=f"mk_{i}") for i in range(16)]
        for i in range(16):
            nc.gpsimd.tensor_scalar(
                mk[i], ramp2, thr2[:, i:i + 1], None, op0=mybir.AluOpType.is_ge)

        with tc.tile_pool(name="psA", bufs=2, space="PSUM") as psA, \
             tc.tile_pool(name="psC", bufs=2, space="PSUM") as psC, \
             tc.tile_pool(name="psO", bufs=2, space="PSUM") as psO:
            ot = [None] * NSLOT

            def emit_pair(j, p):
                pc = psC.tile([128, 1024], f32, tag="pc", name=f"pc_{j}_{p}")
                for d in range(2):
                    kk = 2 * p + d
                    osl = slice(d * 512, d * 512 + 512)
                    qsl = slice(j * 512, j * 512 + 512)
                    ksl = slice(kk * 128, kk * 128 + 128)
                    if USE_FP8:
                        nc.tensor.matmul(
                            pc[:, osl], kTv[:, :, ksl], qTv[:, :, qsl],
                            start=True, stop=True,
                            perf_mode=mybir.MatmulPerfMode.DoubleRow)
                    else:
                        nc.tensor.matmul(pc[:, osl], kTv[:, ksl], qTv[:, qsl],
                                         start=True, stop=True)
                et = epool.tile([128, 2, 512], bf16, tag="et", name=f"et_{j}_{p}")
                nc.scalar.activation(et, pc,
                                     mybir.ActivationFunctionType.Exp,
                                     scale=0.125)
                m = p - (NCH[j] // 2 - 4)
                if m >= 0:
                    nc.vector.tensor_mul(et, et, mk[4 * j + m])
                if p == 0:
                    ot[j] = psO.tile([128, 4, HS + 1], f32, tag="ot",
                                     name=f"ot_{j}")
                for d in range(2):
                    kk = 2 * p + d
                    for qs in range(4):
                        # start zeroes the whole PSUM bank: only the slot's
                        # first matmul may set it; stop only on the last
                        nc.tensor.matmul(
                            ot[j][:, qs, :],
                            et[:, d, qs * 128:qs * 128 + 128],
                            vp[:, kk, :],
                            start=(kk == 0 and qs == 0),
                            stop=(kk == NCH[j] - 1 and qs == 3))
                if p == NCH[j] // 2 - 1:  # finalize slot j
                    for qs in range(4):
                        rec = mpool.tile([128, 1], f32, tag="rec",
                                         name=f"rec_{j}_{qs}")
                        nc.vector.reciprocal(rec, ot[j][:, qs, HS:HS + 1])
                        nc.vector.tensor_scalar_mul(
                            outs[:, 4 * j + qs, :], ot[j][:, qs, 0:HS], rec)
                    nc.sync.dma_start(
                        out=out_d.rearrange("(q p) h -> p q h", p=128)[
                            :, 4 * j:4 * j + 4, :],
                        in_=outs[:, 4 * j:4 * j + 4, :])

            for tb in range(8):
                sl = slice(tb * 512, tb * 512 + 512)
                # A1: K^T|V^T for context block tb
                pa = psA.tile([128, 512], f32, tag="pa", name=f"pa_{tb}")
                for ci in range(CCH):
                    nc.tensor.matmul(pa, wkv[:, ci, :], xt[:, ci, sl],
                                     start=(ci == 0), stop=(ci == CCH - 1))
                if USE_FP8:
                    nc.vector.tensor_copy(kTv[:, 0, sl], pa[0:32, :])
                    nc.vector.tensor_copy(kTv[:, 1, sl], pa[32:64, :])
                else:
                    nc.vector.tensor_copy(kTv[:, sl], pa[0:64, :])
                vts = epool.tile([64, 512], bf16, tag="vts", name=f"vts_{tb}")
                nc.vector.tensor_copy(vts, pa[64:128, :])
                for blk in range(4):
                    kk = tb * 4 + blk
                    vtp = psA.tile([128, HS], bf16, tag="pa", name=f"vtp_{kk}")
                    nc.tensor.transpose(
                        vtp, vts[:, blk * 128:blk * 128 + 128], id_bf)
                    nc.vector.tensor_copy(vp[:, kk, 0:HS], vtp)
                # A2: Q^T for slot tb//2 (own blocks at even positions)
                if tb % 2 == 0:
                    j = tb // 2
                    qsl = slice(j * 512, j * 512 + 512)
                    pq = psA.tile([64, 512], f32, tag="pa", name=f"pq_{j}")
                    for ci in range(CCH):
                        nc.tensor.matmul(pq, wq[:, ci, :], xt[:, ci, sl],
                                         start=(ci == 0), stop=(ci == CCH - 1))
                    if USE_FP8:
                        nc.vector.tensor_copy(qTv[:, 0, qsl], pq[0:32, :])
                        nc.vector.tensor_copy(qTv[:, 1, qsl], pq[32:64, :])
                    else:
                        nc.vector.tensor_copy(qTv[:, qsl], pq[0:64, :])
                for j, p in sched[tb]:
                    emit_pair(j, p)

    nc.compile()
    return nc


def _prep_inputs(x, Wq, Wk, Wv):
    bf = ml_dtypes.bfloat16
    wkv = np.concatenate([Wk, Wv], axis=1).astype(bf)   # [C, 128]
    wq = Wq.astype(bf)
    q = np.arange(512, dtype=np.float32)
    ramp2 = np.concatenate([q, q - 128.0]).reshape(1, 1024)
    ramp2 = np.broadcast_to(ramp2, (128, 1024)).copy()
    p = np.arange(128, dtype=np.float32)
    in_maps = []
    for core in range(8):
        b, h = core // 2, core % 2
        perm = PERM[h]
        xt = np.concatenate(
            [x[b, g * 512:(g + 1) * 512] for g in perm], axis=0
        ).T.astype(bf)
        thr2 = np.zeros((128, 16), np.float32)
        for j in range(NSLOT):
            g = perm[2 * j]
            for pm in range(4):
                kk0 = NCH[j] - 8 + 2 * pm
                base0 = 512 * perm[kk0 // 4] + 128 * (kk0 % 4)
                thr2[:, 4 * j + pm] = base0 + p - 512 * g
        in_maps.append({
            "xt": np.ascontiguousarray(xt),
            "wkv": wkv, "wq": wq, "ramp2": ramp2, "thr2": thr2,
        })
    return in_maps


def kernel(x, Wq, Wk, Wv):
    from concourse.bass_utils import run_bass_kernel_spmd

    global _compiled
    if _compiled is None:
        _compiled = _build_program()
    nc = _compiled

    in_maps = _prep_inputs(
        np.asarray(x, np.float32), np.asarray(Wq, np.float32),
        np.asarray(Wk, np.float32), np.asarray(Wv, np.float32),
    )
    res = run_bass_kernel_spmd(nc, in_maps, list(range(8)))
    out = np.empty((B, T, HS), np.float32)
    for core in range(8):
        b, h = core // 2, core % 2
        perm = PERM[h]
        o = res.results[core]["out"]
        for j in range(NSLOT):
            g = perm[2 * j]
            out[b, g * 512:(g + 1) * 512] = o[j * 512:(j + 1) * 512]
    return out


if __name__ == "__main__":
    rng = np.random.default_rng(0)
    x = rng.standard_normal((B, T, C), dtype=np.float32)
    s = 1 / np.sqrt(C)
    Wq = rng.standard_normal((C, HS), dtype=np.float32) * s
    Wk = rng.standard_normal((C, HS), dtype=np.float32) * s
    Wv = rng.standard_normal((C, HS), dtype=np.float32) * s
    o = kernel(x=x, Wq=Wq, Wk=Wk, Wv=Wv)
    print(o.shape, o.dtype, np.abs(o).mean())


# revision 11
# speedup vs baseline: 1.2787x; 1.0964x over previous
"""Single-head causal self-attention (B=4, T=4096, C=1024, HS=64) on 8 TRN2 cores.

Sharding: core = 2*b + h; the two cores of batch b split the 8 query blocks
(512 rows each) in a load-balanced interleave (h=0 -> {0,3,4,7}, h=1 ->
{1,2,5,6}; 80 causal context chunks each).

The SPMD program is identical on every core; per-core differences are pure
data. Each core's context x[b] is PERMUTED host-side at 512-block granularity
so that the core's own query blocks sit at fixed program positions 0,2,4,6,
while every slot's causal context prefix is covered by the first 2(j+1)
permuted blocks. Causal-mask thresholds (per-core int32 data) absorb the
permutation; the mask ramp is built on-device with iota.

Dataflow per core, wavefront over 16 input granules of 256 columns:
  A1: [K^T|V^T] = [Wk|Wv]^T @ xt granule   (PSUM [128,512] per 512-block)
      V^T -> PE-transpose -> V natural [128k, 64], ones col appended
  A2 (even 512-blocks): Q^T for slot j from the block's own granules
  attention pairs (j, p) drained slot-major as soon as ready:
    C: S^T pair [128k, 2x512q] = K^T.T @ Q^T   (bf16, PSUM 2 banks)
    E: et = exp(0.125 * S^T) (ScalarE, one op per pair), mask last 4 pairs
    D: O[128q, 65] += et_chunk_qslice.T @ [V|1]_chunk  (65-wide moving side)
  finalize per q-tile: rec = 1/O[:,64]; out = O[:,0:64]*rec; DMA out per q-tile
"""

import numpy as np
import ml_dtypes

B, T, C, HS = 4, 4096, 1024, 64
QH = T // 2            # queries per core
NSLOT = 4
NCH = [8, 16, 24, 32]  # uniform context chunks (of 128) per slot
CCH = C // 128
BLOCKS = [[0, 3, 4, 7], [1, 2, 5, 6]]  # own query blocks per half
# permuted context layout: own blocks at positions 0,2,4,6; prefix-coverage
# of each slot's causal context holds for both halves
PERM = [[0, 1, 3, 2, 4, 5, 7, 6], [1, 0, 2, 3, 5, 4, 6, 7]]

_compiled = None


def _attn_schedule():
    """pairs (j, p) per granule g, drained slot-major. A pair is ready once
    qT(j) is copied (g >= 4j+1) and the kT/vp copies for its last chunk's
    512-block are emitted (g >= 2*((2p+1)//4)+1)."""
    work = []
    for j in range(NSLOT):
        for p in range(NCH[j] // 2):
            work.append((j, p))
    sched = [[] for _ in range(16)]
    ptr = 0
    for g in range(16):
        while ptr < len(work):
            j, p = work[ptr]
            if g >= 4 * j + 1 and g >= 2 * ((2 * p + 1) // 4) + 1:
                sched[g].append((j, p))
                ptr += 1
            else:
                break
    assert ptr == len(work)
    return sched


def _build_program():
    import concourse.bass as bass
    import concourse.mybir as mybir
    import concourse.tile as tile
    from concourse import bacc
    from concourse.masks import make_identity
    from contextlib import ExitStack

    f32 = mybir.dt.float32
    bf16 = mybir.dt.bfloat16
    i32 = mybir.dt.int32

    nc = bacc.Bacc("TRN2", target_bir_lowering=False, debug=False, num_devices=8)

    xt_d = nc.dram_tensor("xt", [C, T], bf16, kind="ExternalInput").ap()
    wkv8_d = nc.dram_tensor("wkv8", [128, CCH * 128], bf16,
                            kind="ExternalInput").ap()
    wq8_d = nc.dram_tensor("wq8", [128, CCH * HS], bf16,
                           kind="ExternalInput").ap()
    thr2_d = nc.dram_tensor("thr2", [128, 16], f32, kind="ExternalInput").ap()
    out_d = nc.dram_tensor("out", [QH, HS], f32, kind="ExternalOutput").ap()

    sched = _attn_schedule()

    with tile.TileContext(nc) as tc, ExitStack() as ctx:
        consts = ctx.enter_context(tc.tile_pool(name="consts", bufs=1))
        epool = ctx.enter_context(tc.tile_pool(name="epool", bufs=4))
        mpool = ctx.enter_context(tc.tile_pool(name="mpool", bufs=2))

        xt = consts.tile([128, CCH, T], bf16)
        wkv = consts.tile([128, CCH, 128], bf16)
        wq = consts.tile([128, CCH, HS], bf16)
        ramp2 = consts.tile([128, 2, 512], i32)
        thr2 = consts.tile([128, 16], f32)
        id_bf = consts.tile([64, 64], bf16)
        kTv = consts.tile([64, T], bf16)
        qTv = consts.tile([64, QH], bf16)
        vp = consts.tile([128, T // 128, HS + 1], bf16)  # [V | ones]
        outs = consts.tile([128, QH // 128, HS], f32)

        # DMA order: weights first (A1 needs them), then context granules
        nc.sync.dma_start(out=wkv,
                          in_=wkv8_d.rearrange("p (a m) -> p a m", a=CCH))
        nc.sync.dma_start(out=wq,
                          in_=wq8_d.rearrange("p (a m) -> p a m", a=CCH))
        nc.sync.dma_start(out=thr2, in_=thr2_d)
        xt_r = xt_d.rearrange("(a p) t -> p a t", p=128)
        for g in range(16):
            sl = slice(g * 256, g * 256 + 256)
            nc.sync.dma_start(out=xt[:, :, sl], in_=xt_r[:, :, sl])

        make_identity(nc, id_bf)
        nc.vector.memset(vp[:, :, HS], 1.0)
        # ramp2[p, d, q] = q - 128*d, built on-device (no DMA)
        nc.gpsimd.iota(ramp2, pattern=[[-128, 2], [1, 512]],
                       base=0, channel_multiplier=0)
        # mask pairs on the idle GPSIMD engine, slot-major so slot 0's are
        # ready first
        mk = [consts.tile([128, 2, 512], bf16, name=f"mk_{i}") for i in range(16)]
        for i in range(16):
            nc.gpsimd.tensor_scalar(
                mk[i], ramp2, thr2[:, i:i + 1], None, op0=mybir.AluOpType.is_ge)

        with tc.tile_pool(name="psA", bufs=2, space="PSUM") as psA, \
             tc.tile_pool(name="psC", bufs=2, space="PSUM") as psC, \
             tc.tile_pool(name="psO", bufs=2, space="PSUM") as psO:
            ot = [None] * NSLOT

            def emit_pair(j, p):
                last = p == NCH[j] // 2 - 1
                pc = psC.tile([128, 1024], f32, tag="pc", name=f"pc_{j}_{p}")
                qsl = slice(j * 512, j * 512 + 512)
                for d in range(2):
                    kk = 2 * p + d
                    osl = slice(d * 512, d * 512 + 512)
                    ksl = slice(kk * 128, kk * 128 + 128)
                    nc.tensor.matmul(pc[:, osl], kTv[:, ksl], qTv[:, qsl],
                                     start=True, stop=True)
                et = epool.tile([128, 2, 512], bf16, tag="et", name=f"et_{j}_{p}")
                nc.scalar.activation(et, pc,
                                     mybir.ActivationFunctionType.Exp,
                                     scale=0.125)
                m = p - (NCH[j] // 2 - 4)
                if m >= 0:
                    nc.vector.tensor_mul(et, et, mk[4 * j + m])
                if p == 0:
                    ot[j] = psO.tile([128, 4, HS + 1], f32, tag="ot",
                                     name=f"ot_{j}")

                def dmm(d, qs):
                    kk = 2 * p + d
                    # start zeroes the whole PSUM bank: only the slot's first
                    # matmul sets it; stop ends each q-tile's group
                    nc.tensor.matmul(
                        ot[j][:, qs, :],
                        et[:, d, qs * 128:qs * 128 + 128],
                        vp[:, kk, :],
                        start=(kk == 0 and qs == 0),
                        stop=(kk == NCH[j] - 1 and d == 1 and qs == 3))

                for d in range(2):
                    for qs in range(4):
                        dmm(d, qs)
                if last:  # group closed: finalize + store each q-tile
                    for qs in range(4):
                        rec = mpool.tile([128, 1], f32, tag="rec",
                                         name=f"rec_{j}_{qs}")
                        nc.vector.reciprocal(rec, ot[j][:, qs, HS:HS + 1])
                        nc.vector.tensor_scalar_mul(
                            outs[:, 4 * j + qs, :], ot[j][:, qs, 0:HS], rec)
                        nc.sync.dma_start(
                            out=out_d.rearrange("(q p) h -> p q h", p=128)[
                                :, 4 * j + qs, :],
                            in_=outs[:, 4 * j + qs, :])

            pa_cur = pq_cur = None
            for g in range(16):
                tb, half = g // 2, g % 2
                sl = slice(g * 256, g * 256 + 256)
                hsl = slice(half * 256, half * 256 + 256)
                if half == 0:
                    pa_cur = psA.tile([128, 512], f32, tag="pa",
                                      name=f"pa_{tb}")
                for ci in range(CCH):
                    nc.tensor.matmul(pa_cur[:, hsl], wkv[:, ci, :],
                                     xt[:, ci, sl],
                                     start=(ci == 0 and half == 0),
                                     stop=(ci == CCH - 1 and half == 1))
                if tb % 2 == 0:
                    j = tb // 2
                    if half == 0:
                        pq_cur = psA.tile([64, 512], f32, tag="pa",
                                          name=f"pq_{j}")
                    for ci in range(CCH):
                        nc.tensor.matmul(pq_cur[:, hsl], wq[:, ci, :],
                                         xt[:, ci, sl],
                                         start=(ci == 0 and half == 0),
                                         stop=(ci == CCH - 1 and half == 1))
                if half == 1:
                    # qTv copy must precede the V transposes: they rotate
                    # onto pq's PSUM buffer (tag "pa") and would clobber it
                    if tb % 2 == 0:
                        j = tb // 2
                        nc.vector.tensor_copy(
                            qTv[:, j * 512:j * 512 + 512], pq_cur[0:64, :])
                    bsl = slice(tb * 512, tb * 512 + 512)
                    nc.vector.tensor_copy(kTv[:, bsl], pa_cur[0:64, :])
                    vts = epool.tile([64, 512], bf16, tag="vts",
                                     name=f"vts_{tb}")
                    nc.vector.tensor_copy(vts, pa_cur[64:128, :])
                    for blk in range(4):
                        kk = tb * 4 + blk
                        vtp = psA.tile([128, HS], bf16, tag="pa",
                                       name=f"vtp_{kk}")
                        nc.tensor.transpose(
                            vtp, vts[:, blk * 128:blk * 128 + 128], id_bf)
                        nc.vector.tensor_copy(vp[:, kk, 0:HS], vtp)
                for j, p in sched[g]:
                    emit_pair(j, p)

    nc.compile()
    return nc


def _prep_inputs(x, Wq, Wk, Wv):
    bf = ml_dtypes.bfloat16
    wkv = np.concatenate([Wk, Wv], axis=1)               # [C, 128]
    wkv8 = wkv.reshape(CCH, 128, 128).transpose(1, 0, 2).reshape(128, -1)
    wq8 = Wq.reshape(CCH, 128, HS).transpose(1, 0, 2).reshape(128, -1)
    wkv8 = np.ascontiguousarray(wkv8).astype(bf)
    wq8 = np.ascontiguousarray(wq8).astype(bf)
    p = np.arange(128, dtype=np.int64)
    in_maps = []
    for core in range(8):
        b, h = core // 2, core % 2
        perm = PERM[h]
        xt = np.concatenate(
            [x[b, g * 512:(g + 1) * 512] for g in perm], axis=0
        ).T.astype(bf)
        thr2 = np.zeros((128, 16), np.float32)
        for j in range(NSLOT):
            g = perm[2 * j]
            for pm in range(4):
                kk0 = NCH[j] - 8 + 2 * pm
                base0 = 512 * perm[kk0 // 4] + 128 * (kk0 % 4)
                thr2[:, 4 * j + pm] = base0 + p - 512 * g
        in_maps.append({
            "xt": np.ascontiguousarray(xt),
            "wkv8": wkv8, "wq8": wq8, "thr2": thr2,
        })
    return in_maps


def kernel(x, Wq, Wk, Wv):
    from concourse.bass_utils import run_bass_kernel_spmd

    global _compiled
    if _compiled is None:
        _compiled = _build_program()
    nc = _compiled

    in_maps = _prep_inputs(
        np.asarray(x, np.float32), np.asarray(Wq, np.float32),
        np.asarray(Wk, np.float32), np.asarray(Wv, np.float32),
    )
    res = run_bass_kernel_spmd(nc, in_maps, list(range(8)))
    out = np.empty((B, T, HS), np.float32)
    for core in range(8):
        b, h = core // 2, core % 2
        perm = PERM[h]
        o = res.results[core]["out"]
        for j in range(NSLOT):
            g = perm[2 * j]
            out[b, g * 512:(g + 1) * 512] = o[j * 512:(j + 1) * 512]
    return out


if __name__ == "__main__":
    rng = np.random.default_rng(0)
    x = rng.standard_normal((B, T, C), dtype=np.float32)
    s = 1 / np.sqrt(C)
    Wq = rng.standard_normal((C, HS), dtype=np.float32) * s
    Wk = rng.standard_normal((C, HS), dtype=np.float32) * s
    Wv = rng.standard_normal((C, HS), dtype=np.float32) * s
    o = kernel(x=x, Wq=Wq, Wk=Wk, Wv=Wv)
    print(o.shape, o.dtype, np.abs(o).mean())


# revision 15
# speedup vs baseline: 1.4077x; 1.1009x over previous
"""Single-head causal self-attention (B=4, T=4096, C=1024, HS=64) on 8 TRN2 cores.

Sharding: core = 2*b + h; the two cores of batch b split the 8 query blocks
(512 rows each) in a load-balanced interleave (h=0 -> {0,3,4,7}, h=1 ->
{1,2,5,6}; 80 causal context chunks each).

The SPMD program is identical on every core; per-core differences are pure
data. Each core's context x[b] is PERMUTED host-side at 512-block granularity
so that the core's own query blocks sit at fixed program positions 0,2,4,6,
while every slot's causal context prefix is covered by the first 2(j+1)
permuted blocks. Causal-mask thresholds (per-core int32 data) absorb the
permutation; the mask ramp is built on-device with iota.

Dataflow per core, wavefront over 16 input granules of 256 columns:
  A1: [K^T|V^T] = [Wk|Wv]^T @ xt granule   (PSUM [128,512] per 512-block)
      V^T -> PE-transpose -> V natural [128k, 64], ones col appended
  A2 (even 512-blocks): Q^T for slot j from the block's own granules
  attention pairs (j, p) drained slot-major as soon as ready:
    C: S^T pair [128k, 2x512q] = K^T.T @ Q^T   (bf16, PSUM 2 banks)
    E: et = exp(0.125 * S^T) (ScalarE, one op per pair), mask last 4 pairs
    D: O[128q, 65] += et_chunk_qslice.T @ [V|1]_chunk  (65-wide moving side)
  finalize per q-tile: rec = 1/O[:,64]; out = O[:,0:64]*rec; DMA out per q-tile
"""

import numpy as np
import ml_dtypes

B, T, C, HS = 4, 4096, 1024, 64
QH = T // 2            # queries per core
NSLOT = 4
NCH = [8, 16, 24, 32]  # uniform context chunks (of 128) per slot
CCH = C // 128
BLOCKS = [[0, 3, 4, 7], [1, 2, 5, 6]]  # own query blocks per half
# permuted context layout: own blocks at positions 0,2,4,6; prefix-coverage
# of each slot's causal context holds for both halves
PERM = [[0, 1, 3, 2, 4, 5, 7, 6], [1, 0, 2, 3, 5, 4, 6, 7]]

_compiled = None


def _attn_schedule():
    """C+exp emission (j, p) per granule g, drained slot-major. A pair is
    ready once qT(j) is copied (g >= 4j+1) and the kT/vp copies for its last
    chunk's 512-block are emitted (g >= 2*((2p+1)//4)+1). D matmuls are
    deferred: slot j's D batch is flushed once slot j+1's C stream is rolling
    (g == 4(j+1)+1), keeping the Act engine fed across slot boundaries."""
    work = []
    for j in range(NSLOT):
        for p in range(NCH[j] // 2):
            work.append((j, p))
    sched = [[] for _ in range(16)]
    ptr = 0
    for g in range(16):
        while ptr < len(work):
            j, p = work[ptr]
            if g >= 4 * j + 1 and g >= 2 * ((2 * p + 1) // 4) + 1:
                sched[g].append((j, p))
                ptr += 1
            else:
                break
    assert ptr == len(work)
    return sched


def _build_program():
    import concourse.bass as bass
    import concourse.mybir as mybir
    import concourse.tile as tile
    from concourse import bacc
    from concourse.masks import make_identity
    from contextlib import ExitStack

    f32 = mybir.dt.float32
    bf16 = mybir.dt.bfloat16
    i32 = mybir.dt.int32

    nc = bacc.Bacc("TRN2", target_bir_lowering=False, debug=False, num_devices=8)

    xt_d = nc.dram_tensor("xt", [C, T], bf16, kind="ExternalInput").ap()
    wkv8_d = nc.dram_tensor("wkv8", [128, CCH * 128], bf16,
                            kind="ExternalInput").ap()
    wq8_d = nc.dram_tensor("wq8", [128, CCH * HS], bf16,
                           kind="ExternalInput").ap()
    thr2_d = nc.dram_tensor("thr2", [128, 16], f32, kind="ExternalInput").ap()
    out_d = nc.dram_tensor("out", [QH, HS], f32, kind="ExternalOutput").ap()

    sched = _attn_schedule()

    with tile.TileContext(nc) as tc, ExitStack() as ctx:
        consts = ctx.enter_context(tc.tile_pool(name="consts", bufs=1))
        epool = ctx.enter_context(tc.tile_pool(name="epool", bufs=3))
        mpool = ctx.enter_context(tc.tile_pool(name="mpool", bufs=2))

        xt = consts.tile([128, CCH, T], bf16)
        wkv = consts.tile([128, CCH, 128], bf16)
        wq = consts.tile([128, CCH, HS], bf16)
        ramp2 = consts.tile([128, 2, 512], i32)
        thr2 = consts.tile([128, 16], f32)
        id_bf = consts.tile([64, 64], bf16)
        zsc = consts.tile([64, 512], bf16)
        kTv = consts.tile([64, T], bf16)
        qTv = consts.tile([64, QH], bf16)
        vp = consts.tile([128, T // 128, HS + 1], bf16)  # [V | ones]
        outs = consts.tile([128, QH // 128, HS], f32)

        # DMA order tuned for the critical path: wkv -> granule 0 -> wq ->
        # granule 1 -> thr2 -> remaining granules
        xt_r = xt_d.rearrange("(a p) t -> p a t", p=128)

        def xtg(g):
            sl = slice(g * 256, g * 256 + 256)
            nc.sync.dma_start(out=xt[:, :, sl], in_=xt_r[:, :, sl])

        nc.sync.dma_start(out=wkv,
                          in_=wkv8_d.rearrange("p (a m) -> p a m", a=CCH))
        xtg(0)
        nc.sync.dma_start(out=wq,
                          in_=wq8_d.rearrange("p (a m) -> p a m", a=CCH))
        xtg(1)
        nc.sync.dma_start(out=thr2, in_=thr2_d)
        for g in range(2, 16):
            xtg(g)

        make_identity(nc, id_bf)
        nc.vector.memset(zsc, 0.0)
        nc.vector.memset(vp[:, :, HS], 1.0)
        # ramp2[p, d, q] = q - 128*d, built on-device (no DMA)
        nc.gpsimd.iota(ramp2, pattern=[[-128, 2], [1, 512]],
                       base=0, channel_multiplier=0)
        # mask pairs on the idle GPSIMD engine, slot-major so slot 0's are
        # ready first
        mk = [consts.tile([128, 2, 512], bf16, name=f"mk_{i}") for i in range(16)]
        for i in range(16):
            nc.gpsimd.tensor_scalar(
                mk[i], ramp2, thr2[:, i:i + 1], None, op0=mybir.AluOpType.is_ge)

        with tc.tile_pool(name="psA", bufs=2, space="PSUM") as psA, \
             tc.tile_pool(name="psC", bufs=2, space="PSUM") as psC, \
             tc.tile_pool(name="psO", bufs=2, space="PSUM") as psO:
            ot = [None] * NSLOT
            ets = {}

            # PE warmup: dependency-free matmul chain ramps the p-state
            # clock to full speed before the first projection arrives
            for w in range(13):
                pw = psA.tile([64, 512], f32, tag="pa", name=f"warm_{w}")
                nc.tensor.matmul(pw, id_bf, zsc, start=True, stop=True)

            def emit_ce(j, p):
                pc = psC.tile([128, 1024], f32, tag="pc", name=f"pc_{j}_{p}")
                qsl = slice(j * 512, j * 512 + 512)
                for d in range(2):
                    kk = 2 * p + d
                    osl = slice(d * 512, d * 512 + 512)
                    ksl = slice(kk * 128, kk * 128 + 128)
                    nc.tensor.matmul(pc[:, osl], kTv[:, ksl], qTv[:, qsl],
                                     start=True, stop=True)
                et = epool.tile([128, 2, 512], bf16, tag="et", bufs=28,
                                name=f"et_{j}_{p}")
                nc.scalar.activation(et, pc,
                                     mybir.ActivationFunctionType.Exp,
                                     scale=0.125)
                m = p - (NCH[j] // 2 - 4)
                if m >= 0:
                    nc.vector.tensor_mul(et, et, mk[4 * j + m])
                ets[(j, p)] = et

            def flush_slot(j):
                """emit all deferred D matmuls of slot j, then finalize."""
                ot[j] = psO.tile([128, 4, HS + 1], f32, tag="ot",
                                 name=f"ot_{j}")
                for p in range(NCH[j] // 2):
                    et = ets.pop((j, p))
                    for d in range(2):
                        kk = 2 * p + d
                        for qs in range(4):
                            # start zeroes the whole PSUM bank: only the
                            # slot's first matmul sets it; one stop at the end
                            nc.tensor.matmul(
                                ot[j][:, qs, :],
                                et[:, d, qs * 128:qs * 128 + 128],
                                vp[:, kk, :],
                                start=(kk == 0 and qs == 0),
                                stop=(kk == NCH[j] - 1 and d == 1 and qs == 3))
                for qs in range(4):
                    rec = mpool.tile([128, 1], f32, tag="rec",
                                     name=f"rec_{j}_{qs}")
                    nc.vector.reciprocal(rec, ot[j][:, qs, HS:HS + 1])
                    nc.vector.tensor_scalar_mul(
                        outs[:, 4 * j + qs, :], ot[j][:, qs, 0:HS], rec)
                nc.sync.dma_start(
                    out=out_d.rearrange("(q p) h -> p q h", p=128)[
                        :, 4 * j:4 * j + 4, :],
                    in_=outs[:, 4 * j:4 * j + 4, :])

            pa_cur = pq_cur = None
            for g in range(16):
                tb, half = g // 2, g % 2
                sl = slice(g * 256, g * 256 + 256)
                hsl = slice(half * 256, half * 256 + 256)
                if half == 0:
                    pa_cur = psA.tile([128, 512], f32, tag="pa",
                                      name=f"pa_{tb}")
                for ci in range(CCH):
                    nc.tensor.matmul(pa_cur[:, hsl], wkv[:, ci, :],
                                     xt[:, ci, sl],
                                     start=(ci == 0 and half == 0),
                                     stop=(ci == CCH - 1 and half == 1))
                if tb % 2 == 0:
                    j = tb // 2
                    if half == 0:
                        pq_cur = psA.tile([64, 512], f32, tag="pa",
                                          name=f"pq_{j}")
                    for ci in range(CCH):
                        nc.tensor.matmul(pq_cur[:, hsl], wq[:, ci, :],
                                         xt[:, ci, sl],
                                         start=(ci == 0 and half == 0),
                                         stop=(ci == CCH - 1 and half == 1))
                if half == 1:
                    # qTv copy must precede the V transposes: they rotate
                    # onto pq's PSUM buffer (tag "pa") and would clobber it
                    if tb % 2 == 0:
                        j = tb // 2
                        nc.vector.tensor_copy(
                            qTv[:, j * 512:j * 512 + 512], pq_cur[0:64, :])
                    bsl = slice(tb * 512, tb * 512 + 512)
                    nc.vector.tensor_copy(kTv[:, bsl], pa_cur[0:64, :])
                    vts = epool.tile([64, 512], bf16, tag="vts",
                                     name=f"vts_{tb}")
                    nc.vector.tensor_copy(vts, pa_cur[64:128, :])
                    for blk in range(4):
                        kk = tb * 4 + blk
                        vtp = psA.tile([128, HS], bf16, tag="pa",
                                       name=f"vtp_{kk}")
                        nc.tensor.transpose(
                            vtp, vts[:, blk * 128:blk * 128 + 128], id_bf)
                        nc.vector.tensor_copy(vp[:, kk, 0:HS], vtp)
                for j, p in sched[g]:
                    emit_ce(j, p)
                # slot j's D batch goes behind slot j+1's C stream
                if g >= 5 and g % 4 == 1:
                    flush_slot(g // 4 - 1)
            flush_slot(3)

    nc.compile()
    return nc


def _prep_inputs(x, Wq, Wk, Wv):
    bf = ml_dtypes.bfloat16
    wkv = np.concatenate([Wk, Wv], axis=1)               # [C, 128]
    wkv8 = wkv.reshape(CCH, 128, 128).transpose(1, 0, 2).reshape(128, -1)
    wq8 = Wq.reshape(CCH, 128, HS).transpose(1, 0, 2).reshape(128, -1)
    wkv8 = np.ascontiguousarray(wkv8).astype(bf)
    wq8 = np.ascontiguousarray(wq8).astype(bf)
    p = np.arange(128, dtype=np.int64)
    in_maps = []
    for core in range(8):
        b, h = core // 2, core % 2
        perm = PERM[h]
        xt = np.concatenate(
            [x[b, g * 512:(g + 1) * 512] for g in perm], axis=0
        ).T.astype(bf)
        thr2 = np.zeros((128, 16), np.float32)
        for j in range(NSLOT):
            g = perm[2 * j]
            for pm in range(4):
                kk0 = NCH[j] - 8 + 2 * pm
                base0 = 512 * perm[kk0 // 4] + 128 * (kk0 % 4)
                thr2[:, 4 * j + pm] = base0 + p - 512 * g
        in_maps.append({
            "xt": np.ascontiguousarray(xt),
            "wkv8": wkv8, "wq8": wq8, "thr2": thr2,
        })
    return in_maps


def kernel(x, Wq, Wk, Wv):
    from concourse.bass_utils import run_bass_kernel_spmd

    global _compiled
    if _compiled is None:
        _compiled = _build_program()
    nc = _compiled

    in_maps = _prep_inputs(
        np.asarray(x, np.float32), np.asarray(Wq, np.float32),
        np.asarray(Wk, np.float32), np.asarray(Wv, np.float32),
    )
    res = run_bass_kernel_spmd(nc, in_maps, list(range(8)))
    out = np.empty((B, T, HS), np.float32)
    for core in range(8):
        b, h = core // 2, core % 2
        perm = PERM[h]
        o = res.results[core]["out"]
        for j in range(NSLOT):
            g = perm[2 * j]
            out[b, g * 512:(g + 1) * 512] = o[j * 512:(j + 1) * 512]
    return out


if __name__ == "__main__":
    rng = np.random.default_rng(0)
    x = rng.standard_normal((B, T, C), dtype=np.float32)
    s = 1 / np.sqrt(C)
    Wq = rng.standard_normal((C, HS), dtype=np.float32) * s
    Wk = rng.standard_normal((C, HS), dtype=np.float32) * s
    Wv = rng.standard_normal((C, HS), dtype=np.float32) * s
    o = kernel(x=x, Wq=Wq, Wk=Wk, Wv=Wv)
    print(o.shape, o.dtype, np.abs(o).mean())


# revision 16
# speedup vs baseline: 1.4384x; 1.0218x over previous
"""Single-head causal self-attention (B=4, T=4096, C=1024, HS=64) on 8 TRN2 cores.

Sharding: core = 2*b + h; the two cores of batch b split the 8 query blocks
(512 rows each) in a load-balanced interleave (h=0 -> {0,3,4,7}, h=1 ->
{1,2,5,6}; 80 causal context chunks each).

The SPMD program is identical on every core; per-core differences are pure
data. Each core's context x[b] is PERMUTED host-side at 512-block granularity
so that the core's own query blocks sit at fixed program positions 0,2,4,6,
while every slot's causal context prefix is covered by the first 2(j+1)
permuted blocks. Causal-mask thresholds (per-core int32 data) absorb the
permutation; the mask ramp is built on-device with iota.

Dataflow per core, wavefront over 16 input granules of 256 columns:
  A1: [K^T|V^T] = [Wk|Wv]^T @ xt granule   (PSUM [128,512] per 512-block)
      V^T -> PE-transpose -> V natural [128k, 64], ones col appended
  A2 (even 512-blocks): Q^T for slot j from the block's own granules
  attention pairs (j, p) drained slot-major as soon as ready:
    C: S^T pair [128k, 2x512q] = K^T.T @ Q^T   (bf16, PSUM 2 banks)
    E: et = exp(0.125 * S^T) (ScalarE, one op per pair), mask last 4 pairs
    D: O[128q, 65] += et_chunk_qslice.T @ [V|1]_chunk  (65-wide moving side)
  finalize per q-tile: rec = 1/O[:,64]; out = O[:,0:64]*rec; DMA out per q-tile
"""

import numpy as np
import ml_dtypes

B, T, C, HS = 4, 4096, 1024, 64
QH = T // 2            # queries per core
NSLOT = 4
NCH = [8, 16, 24, 32]  # uniform context chunks (of 128) per slot
CCH = C // 128
BLOCKS = [[0, 3, 4, 7], [1, 2, 5, 6]]  # own query blocks per half
# permuted context layout: own blocks at positions 0,2,4,6; prefix-coverage
# of each slot's causal context holds for both halves
PERM = [[0, 1, 3, 2, 4, 5, 7, 6], [1, 0, 2, 3, 5, 4, 6, 7]]

_compiled = None


# granule issue order: Q-block granule pairs (tb even) pulled forward so
# every slot's exp stream starts as early as possible
GORDER = [0, 1, 4, 5, 2, 3, 8, 9, 6, 7, 12, 13, 10, 11, 14, 15]


def _attn_schedule():
    """C+exp emission (j, p) per position in GORDER. Pair (j, p) is ready
    once its context 512-block (tb = p//2) is fully loaded+copied and qT(j)
    is copied (at the second granule of block 2j). D matmuls are deferred
    per slot (flush_slot) so the Act engine stays fed; flush(j) fires at the
    position where slot j's last CE got emitted."""
    done_pos = {}  # tb -> position where its copies are emitted
    for pos, g in enumerate(GORDER):
        if g % 2 == 1:
            done_pos[g // 2] = pos
    work = []
    for j in range(NSLOT):
        for p in range(NCH[j] // 2):
            work.append((j, p))
    ready = {}
    for j, p in work:
        ready[(j, p)] = max(done_pos[p // 2], done_pos[2 * j])
    sched = [[] for _ in range(16)]
    for j, p in work:
        sched[ready[(j, p)]].append((j, p))
    # tail polish: end slot 3 on unmasked pairs so the post-exp chain of the
    # very last pair skips the mask multiply
    lastpos = max(ready[(3, p)] for p in range(NCH[3] // 2))
    tailset = [(3, p) for p in (10, 11)]
    for pr in tailset:
        if pr in sched[lastpos]:
            tailset = []  # already last; leave as-is
            break
    for pr in tailset:
        for s in sched:
            if pr in s:
                s.remove(pr)
        sched[lastpos].append(pr)
    flush = [[] for _ in range(16)]
    for j in range(NSLOT):
        fpos = max(ready[(j, p)] for p in range(NCH[j] // 2))
        if j == 3:
            fpos = 15
        flush[max(fpos, done_pos[2 * j])].append(j)
    return sched, flush


def _build_program():
    import concourse.bass as bass
    import concourse.mybir as mybir
    import concourse.tile as tile
    from concourse import bacc
    from concourse.masks import make_identity
    from contextlib import ExitStack

    f32 = mybir.dt.float32
    bf16 = mybir.dt.bfloat16
    i32 = mybir.dt.int32

    nc = bacc.Bacc("TRN2", target_bir_lowering=False, debug=False, num_devices=8)

    xt_d = nc.dram_tensor("xt", [C, T], bf16, kind="ExternalInput").ap()
    wkv8_d = nc.dram_tensor("wkv8", [128, CCH * 128], bf16,
                            kind="ExternalInput").ap()
    wq8_d = nc.dram_tensor("wq8", [128, CCH * HS], bf16,
                           kind="ExternalInput").ap()
    thr2_d = nc.dram_tensor("thr2", [128, 16], f32, kind="ExternalInput").ap()
    out_d = nc.dram_tensor("out", [QH, HS], f32, kind="ExternalOutput").ap()

    sched, flush = _attn_schedule()

    with tile.TileContext(nc) as tc, ExitStack() as ctx:
        consts = ctx.enter_context(tc.tile_pool(name="consts", bufs=1))
        epool = ctx.enter_context(tc.tile_pool(name="epool", bufs=3))
        mpool = ctx.enter_context(tc.tile_pool(name="mpool", bufs=2))

        xt = consts.tile([128, CCH, T], bf16)
        wkv = consts.tile([128, CCH, 128], bf16)
        wq = consts.tile([128, CCH, HS], bf16)
        ramp2 = consts.tile([128, 2, 512], i32)
        thr2 = consts.tile([128, 16], f32)
        id_bf = consts.tile([64, 64], bf16)
        zsc = consts.tile([64, 512], bf16)
        kTv = consts.tile([64, T], bf16)
        qTv = consts.tile([64, QH], bf16)
        vp = consts.tile([128, T // 128, HS + 1], bf16)  # [V | ones]
        outs = consts.tile([128, QH // 128, HS], f32)

        # DMA order tuned for the critical path: wkv -> granule 0 -> wq ->
        # granule 1 -> thr2 -> remaining granules
        xt_r = xt_d.rearrange("(a p) t -> p a t", p=128)

        def xtg(g):
            sl = slice(g * 256, g * 256 + 256)
            nc.sync.dma_start(out=xt[:, :, sl], in_=xt_r[:, :, sl])

        nc.sync.dma_start(out=wkv,
                          in_=wkv8_d.rearrange("p (a m) -> p a m", a=CCH))
        xtg(GORDER[0])
        nc.sync.dma_start(out=wq,
                          in_=wq8_d.rearrange("p (a m) -> p a m", a=CCH))
        xtg(GORDER[1])
        nc.sync.dma_start(out=thr2, in_=thr2_d)
        for pos in range(2, 16):
            xtg(GORDER[pos])

        make_identity(nc, id_bf)
        nc.vector.memset(zsc, 0.0)
        nc.vector.memset(vp[:, :, HS], 1.0)
        # ramp2[p, d, q] = q - 128*d, built on-device (no DMA)
        nc.gpsimd.iota(ramp2, pattern=[[-128, 2], [1, 512]],
                       base=0, channel_multiplier=0)
        # mask pairs on the idle GPSIMD engine, slot-major so slot 0's are
        # ready first
        mk = [consts.tile([128, 2, 512], bf16, name=f"mk_{i}") for i in range(16)]
        for i in range(16):
            nc.gpsimd.tensor_scalar(
                mk[i], ramp2, thr2[:, i:i + 1], None, op0=mybir.AluOpType.is_ge)

        with tc.tile_pool(name="psA", bufs=2, space="PSUM") as psA, \
             tc.tile_pool(name="psC", bufs=2, space="PSUM") as psC, \
             tc.tile_pool(name="psO", bufs=2, space="PSUM") as psO:
            ot = [None] * NSLOT
            ets = {}

            # PE warmup: dependency-free matmul chain ramps the p-state
            # clock to full speed before the first projection arrives
            for w in range(13):
                pw = psA.tile([64, 512], f32, tag="pa", name=f"warm_{w}")
                nc.tensor.matmul(pw, id_bf, zsc, start=True, stop=True)

            def emit_ce(j, p):
                pc = psC.tile([128, 1024], f32, tag="pc", name=f"pc_{j}_{p}")
                qsl = slice(j * 512, j * 512 + 512)
                for d in range(2):
                    kk = 2 * p + d
                    osl = slice(d * 512, d * 512 + 512)
                    ksl = slice(kk * 128, kk * 128 + 128)
                    nc.tensor.matmul(pc[:, osl], kTv[:, ksl], qTv[:, qsl],
                                     start=True, stop=True)
                et = epool.tile([128, 2, 512], bf16, tag="et", bufs=28,
                                name=f"et_{j}_{p}")
                nc.scalar.activation(et, pc,
                                     mybir.ActivationFunctionType.Exp,
                                     scale=0.125)
                m = p - (NCH[j] // 2 - 4)
                if m >= 0:
                    nc.vector.tensor_mul(et, et, mk[4 * j + m])
                ets[(j, p)] = et

            def flush_slot(j):
                """emit all deferred D matmuls of slot j, then finalize."""
                ot[j] = psO.tile([128, 4, HS + 1], f32, tag="ot",
                                 name=f"ot_{j}")
                for p in range(NCH[j] // 2):
                    et = ets.pop((j, p))
                    for d in range(2):
                        kk = 2 * p + d
                        for qs in range(4):
                            # start zeroes the whole PSUM bank: only the
                            # slot's first matmul sets it; one stop at the end
                            nc.tensor.matmul(
                                ot[j][:, qs, :],
                                et[:, d, qs * 128:qs * 128 + 128],
                                vp[:, kk, :],
                                start=(kk == 0 and qs == 0),
                                stop=(kk == NCH[j] - 1 and d == 1 and qs == 3))
                for qs in range(4):
                    rec = mpool.tile([128, 1], f32, tag="rec",
                                     name=f"rec_{j}_{qs}")
                    nc.vector.reciprocal(rec, ot[j][:, qs, HS:HS + 1])
                    nc.vector.tensor_scalar_mul(
                        outs[:, 4 * j + qs, :], ot[j][:, qs, 0:HS], rec)
                nc.sync.dma_start(
                    out=out_d.rearrange("(q p) h -> p q h", p=128)[
                        :, 4 * j:4 * j + 4, :],
                    in_=outs[:, 4 * j:4 * j + 4, :])

            pa_cur = pq_cur = None
            for pos in range(16):
                g = GORDER[pos]
                tb, half = g // 2, g % 2
                sl = slice(g * 256, g * 256 + 256)
                hsl = slice(half * 256, half * 256 + 256)
                if half == 0:
                    pa_cur = psA.tile([128, 512], f32, tag="pa",
                                      name=f"pa_{tb}")
                for ci in range(CCH):
                    nc.tensor.matmul(pa_cur[:, hsl], wkv[:, ci, :],
                                     xt[:, ci, sl],
                                     start=(ci == 0 and half == 0),
                                     stop=(ci == CCH - 1 and half == 1))
                if tb % 2 == 0:
                    j = tb // 2
                    if half == 0:
                        pq_cur = psA.tile([64, 512], f32, tag="pa",
                                          name=f"pq_{j}")
                    for ci in range(CCH):
                        nc.tensor.matmul(pq_cur[:, hsl], wq[:, ci, :],
                                         xt[:, ci, sl],
                                         start=(ci == 0 and half == 0),
                                         stop=(ci == CCH - 1 and half == 1))
                if half == 1:
                    # qTv copy must precede the V transposes: they rotate
                    # onto pq's PSUM buffer (tag "pa") and would clobber it
                    if tb % 2 == 0:
                        j = tb // 2
                        nc.vector.tensor_copy(
                            qTv[:, j * 512:j * 512 + 512], pq_cur[0:64, :])
                    bsl = slice(tb * 512, tb * 512 + 512)
                    nc.vector.tensor_copy(kTv[:, bsl], pa_cur[0:64, :])
                    vts = epool.tile([64, 512], bf16, tag="vts",
                                     name=f"vts_{tb}")
                    nc.vector.tensor_copy(vts, pa_cur[64:128, :])
                    for blk in range(4):
                        kk = tb * 4 + blk
                        vtp = psA.tile([128, HS], bf16, tag="pa",
                                       name=f"vtp_{kk}")
                        nc.tensor.transpose(
                            vtp, vts[:, blk * 128:blk * 128 + 128], id_bf)
                        nc.vector.tensor_copy(vp[:, kk, 0:HS], vtp)
                for j, p in sched[pos]:
                    emit_ce(j, p)
                for j in flush[pos]:
                    flush_slot(j)

    nc.compile()
    return nc


def _prep_inputs(x, Wq, Wk, Wv):
    bf = ml_dtypes.bfloat16
    wkv = np.concatenate([Wk, Wv], axis=1)               # [C, 128]
    wkv8 = wkv.reshape(CCH, 128, 128).transpose(1, 0, 2).reshape(128, -1)
    wq8 = Wq.reshape(CCH, 128, HS).transpose(1, 0, 2).reshape(128, -1)
    wkv8 = np.ascontiguousarray(wkv8).astype(bf)
    wq8 = np.ascontiguousarray(wq8).astype(bf)
    p = np.arange(128, dtype=np.int64)
    in_maps = []
    for core in range(8):
        b, h = core // 2, core % 2
        perm = PERM[h]
        xt = np.concatenate(
            [x[b, g * 512:(g + 1) * 512] for g in perm], axis=0
        ).T.astype(bf)
        thr2 = np.zeros((128, 16), np.float32)
        for j in range(NSLOT):
            g = perm[2 * j]
            for pm in range(4):
                kk0 = NCH[j] - 8 + 2 * pm
                base0 = 512 * perm[kk0 // 4] + 128 * (kk0 % 4)
                thr2[:, 4 * j + pm] = base0 + p - 512 * g
        in_maps.append({
            "xt": np.ascontiguousarray(xt),
            "wkv8": wkv8, "wq8": wq8, "thr2": thr2,
        })
    return in_maps


def kernel(x, Wq, Wk, Wv):
    from concourse.bass_utils import run_bass_kernel_spmd

    global _compiled
    if _compiled is None:
        _compiled = _build_program()
    nc = _compiled

    in_maps = _prep_inputs(
        np.asarray(x, np.float32), np.asarray(Wq, np.float32),
        np.asarray(Wk, np.float32), np.asarray(Wv, np.float32),
    )
    res = run_bass_kernel_spmd(nc, in_maps, list(range(8)))
    out = np.empty((B, T, HS), np.float32)
    for core in range(8):
        b, h = core // 2, core % 2
        perm = PERM[h]
        o = res.results[core]["out"]
        for j in range(NSLOT):
            g = perm[2 * j]
            out[b, g * 512:(g + 1) * 512] = o[j * 512:(j + 1) * 512]
    return out


if __name__ == "__main__":
    rng = np.random.default_rng(0)
    x = rng.standard_normal((B, T, C), dtype=np.float32)
    s = 1 / np.sqrt(C)
    Wq = rng.standard_normal((C, HS), dtype=np.float32) * s
    Wk = rng.standard_normal((C, HS), dtype=np.float32) * s
    Wv = rng.standard_normal((C, HS), dtype=np.float32) * s
    o = kernel(x=x, Wq=Wq, Wk=Wk, Wv=Wv)
    print(o.shape, o.dtype, np.abs(o).mean())


# revision 17
# speedup vs baseline: 1.4690x; 1.0213x over previous
"""Single-head causal self-attention (B=4, T=4096, C=1024, HS=64) on 8 TRN2 cores.

Sharding: core = 2*b + h; the two cores of batch b split the 8 query blocks
(512 rows each) in a load-balanced interleave (h=0 -> {0,3,4,7}, h=1 ->
{1,2,5,6}; 80 causal context chunks each).

The SPMD program is identical on every core; per-core differences are pure
data. Each core's context x[b] is PERMUTED host-side at 512-block granularity
so that the core's own query blocks sit at fixed program positions 0,2,4,6,
while every slot's causal context prefix is covered by the first 2(j+1)
permuted blocks. Causal-mask thresholds (per-core int32 data) absorb the
permutation; the mask ramp is built on-device with iota.

Dataflow per core, wavefront over 16 input granules of 256 columns:
  A1: [K^T|V^T] = [Wk|Wv]^T @ xt granule   (PSUM [128,512] per 512-block)
      V^T -> PE-transpose -> V natural [128k, 64], ones col appended
  A2 (even 512-blocks): Q^T for slot j from the block's own granules
  attention pairs (j, p) drained slot-major as soon as ready:
    C: S^T pair [128k, 2x512q] = K^T.T @ Q^T   (bf16, PSUM 2 banks)
    E: et = exp(0.125 * S^T) (ScalarE, one op per pair), mask last 4 pairs
    D: O[128q, 65] += et_chunk_qslice.T @ [V|1]_chunk  (65-wide moving side)
  finalize per q-tile: rec = 1/O[:,64]; out = O[:,0:64]*rec; DMA out per q-tile
"""

import numpy as np
import ml_dtypes

B, T, C, HS = 4, 4096, 1024, 64
QH = T // 2            # queries per core
NSLOT = 4
NCH = [8, 16, 24, 32]  # uniform context chunks (of 128) per slot
CCH = C // 128
BLOCKS = [[0, 3, 4, 7], [1, 2, 5, 6]]  # own query blocks per half
# permuted context layout: own blocks at positions 0,2,4,6; prefix-coverage
# of each slot's causal context holds for both halves
PERM = [[0, 1, 3, 2, 4, 5, 7, 6], [1, 0, 2, 3, 5, 4, 6, 7]]

_compiled = None


# granule issue order: Q-block granule pairs (tb even) pulled forward so
# every slot's exp stream starts as early as possible
GORDER = [0, 1, 4, 5, 2, 3, 8, 9, 6, 7, 12, 13, 10, 11, 14, 15]


def _attn_schedule():
    """C+exp emission (j, p) per position in GORDER. Pair (j, p) is ready
    once its context 512-block (tb = p//2) is fully loaded+copied and qT(j)
    is copied (at the second granule of block 2j). D matmuls are deferred
    per slot (flush_slot) so the Act engine stays fed; flush(j) fires at the
    position where slot j's last CE got emitted."""
    done_pos = {}  # tb -> position where its copies are emitted
    for pos, g in enumerate(GORDER):
        if g % 2 == 1:
            done_pos[g // 2] = pos
    work = []
    for j in range(NSLOT):
        for p in range(NCH[j] // 2):
            work.append((j, p))
    ready = {}
    for j, p in work:
        ready[(j, p)] = max(done_pos[p // 2], done_pos[2 * j])
    sched = [[] for _ in range(16)]
    for j, p in work:
        sched[ready[(j, p)]].append((j, p))
    # tail polish: end slot 3 on unmasked pairs so the post-exp chain of the
    # very last pair skips the mask multiply
    lastpos = max(ready[(3, p)] for p in range(NCH[3] // 2))
    tailset = [(3, 11)]
    for pr in tailset:
        if pr in sched[lastpos]:
            tailset = []  # already last; leave as-is
            break
    for pr in tailset:
        for s in sched:
            if pr in s:
                s.remove(pr)
        sched[lastpos].append(pr)
    # spread D batches (PE filler) across positions; fin=True on a slot's
    # final batch. Slot 3's tail pairs go last so the post-exp chain of the
    # final pair (3,11) is minimal.
    dplan = {
        5: [(0, [0, 1, 2, 3], True)],
        7: [(1, [0, 1, 2, 3], False)],
        9: [(1, [4, 5], False)],
        11: [(1, [6, 7], True), (2, [0, 1, 2, 3], False)],
        12: [(2, [4, 5, 6, 7], False)],
        13: [(2, [8, 9], False), (3, [0, 1], False)],
        14: [(3, [2, 3, 4, 5, 6, 7], False)],
        15: [(2, [10, 11], True),
             (3, [8, 9, 12, 13, 10, 14, 15, 11], True)],
    }
    emitted = [[] for _ in range(NSLOT)]
    for pos in sorted(dplan):
        for j, ps, fin in dplan[pos]:
            for p in ps:
                assert ready[(j, p)] <= pos or (j, p) in [(3, 11)], (j, p, pos)
                emitted[j].append(p)
    for j in range(NSLOT):
        assert sorted(emitted[j]) == list(range(NCH[j] // 2)), j
    return sched, dplan


def _build_program():
    import concourse.bass as bass
    import concourse.mybir as mybir
    import concourse.tile as tile
    from concourse import bacc
    from concourse.masks import make_identity
    from contextlib import ExitStack

    f32 = mybir.dt.float32
    bf16 = mybir.dt.bfloat16
    i32 = mybir.dt.int32

    nc = bacc.Bacc("TRN2", target_bir_lowering=False, debug=False, num_devices=8)

    xt_d = nc.dram_tensor("xt", [C, T], bf16, kind="ExternalInput").ap()
    wkv8_d = nc.dram_tensor("wkv8", [128, CCH * 128], bf16,
                            kind="ExternalInput").ap()
    wq8_d = nc.dram_tensor("wq8", [128, CCH * HS], bf16,
                           kind="ExternalInput").ap()
    thr2_d = nc.dram_tensor("thr2", [128, 16], f32, kind="ExternalInput").ap()
    out_d = nc.dram_tensor("out", [QH, HS], f32, kind="ExternalOutput").ap()

    sched, dplan = _attn_schedule()

    with tile.TileContext(nc) as tc, ExitStack() as ctx:
        consts = ctx.enter_context(tc.tile_pool(name="consts", bufs=1))
        epool = ctx.enter_context(tc.tile_pool(name="epool", bufs=3))
        mpool = ctx.enter_context(tc.tile_pool(name="mpool", bufs=2))

        xt = consts.tile([128, CCH, T], bf16)
        wkv = consts.tile([128, CCH, 128], bf16)
        wq = consts.tile([128, CCH, HS], bf16)
        ramp2 = consts.tile([128, 2, 512], i32)
        thr2 = consts.tile([128, 16], f32)
        id_bf = consts.tile([64, 64], bf16)
        zsc = consts.tile([64, 512], bf16)
        kTv = consts.tile([64, T], bf16)
        qTv = consts.tile([64, QH], bf16)
        vp = consts.tile([128, T // 128, HS + 1], bf16)  # [V | ones]
        outs = consts.tile([128, QH // 128, HS], f32)

        # DMA order tuned for the critical path: wkv -> granule 0 -> wq ->
        # granule 1 -> thr2 -> remaining granules
        xt_r = xt_d.rearrange("(a p) t -> p a t", p=128)

        def xtg(g):
            sl = slice(g * 256, g * 256 + 256)
            nc.sync.dma_start(out=xt[:, :, sl], in_=xt_r[:, :, sl])

        nc.sync.dma_start(out=wkv,
                          in_=wkv8_d.rearrange("p (a m) -> p a m", a=CCH))
        xtg(GORDER[0])
        nc.sync.dma_start(out=wq,
                          in_=wq8_d.rearrange("p (a m) -> p a m", a=CCH))
        xtg(GORDER[1])
        nc.sync.dma_start(out=thr2, in_=thr2_d)
        for pos in range(2, 16):
            xtg(GORDER[pos])

        nc.gpsimd.memset(zsc, 0.0)
        make_identity(nc, id_bf)
        nc.vector.memset(vp[:, :, HS], 1.0)
        # ramp2[p, d, q] = q - 128*d, built on-device (no DMA)
        nc.gpsimd.iota(ramp2, pattern=[[-128, 2], [1, 512]],
                       base=0, channel_multiplier=0)
        # mask pairs on the idle GPSIMD engine, slot-major so slot 0's are
        # ready first
        mk = [consts.tile([128, 2, 512], bf16, name=f"mk_{i}") for i in range(16)]
        for i in range(16):
            nc.gpsimd.tensor_scalar(
                mk[i], ramp2, thr2[:, i:i + 1], None, op0=mybir.AluOpType.is_ge)

        with tc.tile_pool(name="psA", bufs=2, space="PSUM") as psA, \
             tc.tile_pool(name="psC", bufs=2, space="PSUM") as psC, \
             tc.tile_pool(name="psO", bufs=2, space="PSUM") as psO:
            ot = [None] * NSLOT
            ets = {}

            # PE warmup: dependency-free matmul chain ramps the p-state
            # clock to full speed before the first projection arrives
            for w in range(11):
                pw = psA.tile([64, 512], f32, tag="pa", name=f"warm_{w}")
                nc.tensor.matmul(pw, id_bf, zsc, start=True, stop=True)

            def emit_ce(j, p):
                pc = psC.tile([128, 1024], f32, tag="pc", name=f"pc_{j}_{p}")
                qsl = slice(j * 512, j * 512 + 512)
                for d in range(2):
                    kk = 2 * p + d
                    osl = slice(d * 512, d * 512 + 512)
                    ksl = slice(kk * 128, kk * 128 + 128)
                    nc.tensor.matmul(pc[:, osl], kTv[:, ksl], qTv[:, qsl],
                                     start=True, stop=True)
                et = epool.tile([128, 2, 512], bf16, tag="et", bufs=28,
                                name=f"et_{j}_{p}")
                nc.scalar.activation(et, pc,
                                     mybir.ActivationFunctionType.Exp,
                                     scale=0.125)
                m = p - (NCH[j] // 2 - 4)
                if m >= 0:
                    nc.vector.tensor_mul(et, et, mk[4 * j + m])
                ets[(j, p)] = et

            def flush_pairs(j, ps, fin):
                """emit deferred D matmuls for pairs ps of slot j; finalize
                when fin (ps then contains the slot's last emitted pair)."""
                if ot[j] is None:
                    ot[j] = psO.tile([128, 4, HS + 1], f32, tag="ot",
                                     name=f"ot_{j}")
                for i, p in enumerate(ps):
                    first = p == 0
                    last = fin and i == len(ps) - 1
                    et = ets.pop((j, p))
                    for d in range(2):
                        kk = 2 * p + d
                        for qs in range(4):
                            # start zeroes the whole PSUM bank: set only on
                            # the slot's first emitted matmul; one stop on
                            # the last emitted one
                            nc.tensor.matmul(
                                ot[j][:, qs, :],
                                et[:, d, qs * 128:qs * 128 + 128],
                                vp[:, kk, :],
                                start=(first and d == 0 and qs == 0),
                                stop=(last and d == 1 and qs == 3))
                if fin:
                    for qs in range(4):
                        rec = mpool.tile([128, 1], f32, tag="rec",
                                         name=f"rec_{j}_{qs}")
                        nc.vector.reciprocal(rec, ot[j][:, qs, HS:HS + 1])
                        nc.vector.tensor_scalar_mul(
                            outs[:, 4 * j + qs, :], ot[j][:, qs, 0:HS], rec)
                    nc.sync.dma_start(
                        out=out_d.rearrange("(q p) h -> p q h", p=128)[
                            :, 4 * j:4 * j + 4, :],
                        in_=outs[:, 4 * j:4 * j + 4, :])

            pa_cur = pq_cur = None
            for pos in range(16):
                g = GORDER[pos]
                tb, half = g // 2, g % 2
                sl = slice(g * 256, g * 256 + 256)
                hsl = slice(half * 256, half * 256 + 256)
                if half == 0:
                    pa_cur = psA.tile([128, 512], f32, tag="pa",
                                      name=f"pa_{tb}")
                for ci in range(CCH):
                    nc.tensor.matmul(pa_cur[:, hsl], wkv[:, ci, :],
                                     xt[:, ci, sl],
                                     start=(ci == 0 and half == 0),
                                     stop=(ci == CCH - 1 and half == 1))
                if tb % 2 == 0:
                    j = tb // 2
                    if half == 0:
                        pq_cur = psA.tile([64, 512], f32, tag="pa",
                                          name=f"pq_{j}")
                    for ci in range(CCH):
                        nc.tensor.matmul(pq_cur[:, hsl], wq[:, ci, :],
                                         xt[:, ci, sl],
                                         start=(ci == 0 and half == 0),
                                         stop=(ci == CCH - 1 and half == 1))
                if half == 1:
                    # qTv copy must precede the V transposes: they rotate
                    # onto pq's PSUM buffer (tag "pa") and would clobber it
                    if tb % 2 == 0:
                        j = tb // 2
                        nc.vector.tensor_copy(
                            qTv[:, j * 512:j * 512 + 512], pq_cur[0:64, :])
                    bsl = slice(tb * 512, tb * 512 + 512)
                    nc.vector.tensor_copy(kTv[:, bsl], pa_cur[0:64, :])
                    vts = epool.tile([64, 512], bf16, tag="vts",
                                     name=f"vts_{tb}")
                    nc.vector.tensor_copy(vts, pa_cur[64:128, :])
                    for blk in range(4):
                        kk = tb * 4 + blk
                        vtp = psA.tile([128, HS], bf16, tag="pa",
                                       name=f"vtp_{kk}")
                        nc.tensor.transpose(
                            vtp, vts[:, blk * 128:blk * 128 + 128], id_bf)
                        nc.vector.tensor_copy(vp[:, kk, 0:HS], vtp)
                for j, p in sched[pos]:
                    emit_ce(j, p)
                for j, ps, fin in dplan.get(pos, []):
                    flush_pairs(j, ps, fin)

    nc.compile()
    return nc


def _prep_inputs(x, Wq, Wk, Wv):
    bf = ml_dtypes.bfloat16
    wkv = np.concatenate([Wk, Wv], axis=1)               # [C, 128]
    wkv8 = wkv.reshape(CCH, 128, 128).transpose(1, 0, 2).reshape(128, -1)
    wq8 = Wq.reshape(CCH, 128, HS).transpose(1, 0, 2).reshape(128, -1)
    wkv8 = np.ascontiguousarray(wkv8).astype(bf)
    wq8 = np.ascontiguousarray(wq8).astype(bf)
    p = np.arange(128, dtype=np.int64)
    in_maps = []
    for core in range(8):
        b, h = core // 2, core % 2
        perm = PERM[h]
        xt = np.concatenate(
            [x[b, g * 512:(g + 1) * 512] for g in perm], axis=0
        ).T.astype(bf)
        thr2 = np.zeros((128, 16), np.float32)
        for j in range(NSLOT):
            g = perm[2 * j]
            for pm in range(4):
                kk0 = NCH[j] - 8 + 2 * pm
                base0 = 512 * perm[kk0 // 4] + 128 * (kk0 % 4)
                thr2[:, 4 * j + pm] = base0 + p - 512 * g
        in_maps.append({
            "xt": np.ascontiguousarray(xt),
            "wkv8": wkv8, "wq8": wq8, "thr2": thr2,
        })
    return in_maps


def kernel(x, Wq, Wk, Wv):
    from concourse.bass_utils import run_bass_kernel_spmd

    global _compiled
    if _compiled is None:
        _compiled = _build_program()
    nc = _compiled

    in_maps = _prep_inputs(
        np.asarray(x, np.float32), np.asarray(Wq, np.float32),
        np.asarray(Wk, np.float32), np.asarray(Wv, np.float32),
    )
    res = run_bass_kernel_spmd(nc, in_maps, list(range(8)))
    out = np.empty((B, T, HS), np.float32)
    for core in range(8):
        b, h = core // 2, core % 2
        perm = PERM[h]
        o = res.results[core]["out"]
        for j in range(NSLOT):
            g = perm[2 * j]
            out[b, g * 512:(g + 1) * 512] = o[j * 512:(j + 1) * 512]
    return out


if __name__ == "__main__":
    rng = np.random.default_rng(0)
    x = rng.standard_normal((B, T, C), dtype=np.float32)
    s = 1 / np.sqrt(C)
    Wq = rng.standard_normal((C, HS), dtype=np.float32) * s
    Wk = rng.standard_normal((C, HS), dtype=np.float32) * s
    Wv = rng.standard_normal((C, HS), dtype=np.float32) * s
    o = kernel(x=x, Wq=Wq, Wk=Wk, Wv=Wv)
    print(o.shape, o.dtype, np.abs(o).mean())
